# revision 1
# baseline (speedup 1.0000x reference)
"""Trainium2 Bass kernel for nn_ConnectFourFeatures.

Computes out = concat([x, conv(x, f) for f in 8 filters], axis=1) where each
filter is channel-diagonal with 0/1 taps along a line (horiz/vert/diag/anti,
run lengths 3 and 5), on inputs x of shape [131072, 3, 6, 7] fp32.

Strategy: pure data-parallel across 8 NeuronCores (batch sharded). Per core,
batch-tile x into SBUF as [128 partitions, spt samples, 3, 6, 7]; each conv is
a base copy (ScalarE) plus two clipped shifted tensor_add ops (VectorE); the
l=5 filters reuse the l=3 result (their taps are a superset). One big output
tile [128, spt, 27, 6, 7] per batch-tile is DMA'd back contiguously.
"""

import numpy as np

import concourse.bass as bass
import concourse.mybir as mybir
from concourse import bacc
from concourse.bass_utils import run_bass_kernel_spmd
from concourse.tile import TileContext

N_CORES = 8
H, W = 6, 7
CIN = 3
FX = CIN * H * W  # 126
COUT = 27
FO = COUT * H * W  # 1134
P = 128
DIRS = [(0, 1), (1, 0), (1, 1), (1, -1)]  # horiz, vert, diag, anti
F32 = mybir.dt.float32


def _shift_slices(dh, dw):
    """Regions for out[h,w] += x[h+dh, w+dw] with clipping."""
    hs = slice(max(0, -dh), H - max(0, dh))
    ws = slice(max(0, -dw), W - max(0, dw))
    hs2 = slice(max(0, dh), H - max(0, -dh))
    ws2 = slice(max(0, dw), W - max(0, -dw))
    return (hs, ws), (hs2, ws2)


def _region(dh, dw):
    """Valid (h, w) slices for reading x[h+dh, w+dw]."""
    hs = slice(max(0, -dh), H - max(0, dh))
    ws = slice(max(0, -dw), W - max(0, dw))
    return hs, ws


def _shifted(hs, ws, dh, dw):
    return slice(hs.start + dh, hs.stop + dh), slice(ws.start + dw, ws.stop + dw)


def _isect(a, b):
    return slice(max(a.start, b.start), min(a.stop, b.stop))


def _emit_add(nc, out_blk, out_hs, out_ws, in_blk, in_hs, in_ws, in_place):
    """out_blk[:, :, :, out_hs, out_ws] (+)= in_blk[..., in_hs, in_ws] over all
    CIN channels; splits per channel unless one of h/w is a full range."""
    full_h = (out_hs.start, out_hs.stop) == (0, H) and (in_hs.start, in_hs.stop) == (0, H)
    full_w = (out_ws.start, out_ws.stop) == (0, W) and (in_ws.start, in_ws.stop) == (0, W)
    def one(o, i0, i1):
        nc.vector.tensor_add(out=o, in0=i0, in1=i1)
    if full_h or full_w:
        o = out_blk[:, :, :, out_hs, out_ws]
        i1 = in_blk[:, :, :, in_hs, in_ws]
        i0 = o if in_place else None
        if in_place:
            nc.vector.tensor_add(out=o, in0=o, in1=i1)
        else:
            raise AssertionError("non-inplace merged adds handled by caller")
    else:
        for c in range(CIN):
            o = out_blk[:, :, c, out_hs, out_ws]
            i1 = in_blk[:, :, c, in_hs, in_ws]
            nc.vector.tensor_add(out=o, in0=o, in1=i1)


def build_nc(n_samples, spt=16, x_bufs=4, o_bufs=2, split_out_dma=True, repeats=1):
    """Build the per-core Bass program: x [n_samples, 126] -> out [n_samples, 1134].

    l=3 filters: DVE-only pair-add scheme (no base copy):
      op1: out[R+] = x[R+] + x[R+ shifted +d]
      op2: out[comp(R+) & R-] = x + x[-d]   (+ corner copies for diagonals)
      op3: out[R+ & R-] += x[-d]
    l=5 filters: ACT base copy of the l=3 block + two in-place DVE adds (+-2d).
    """
    tile_samples = P * spt
    assert n_samples % tile_samples == 0, (n_samples, tile_samples)
    nt = n_samples // tile_samples

    nc = bacc.Bacc(None, target_bir_lowering=False)
    x_d = nc.dram_tensor("x", [n_samples, FX], F32, kind="ExternalInput")
    o_d = nc.dram_tensor("out", [n_samples, FO], F32, kind="ExternalOutput")

    with TileContext(nc) as tc:
        with (
            tc.tile_pool(name="xp", bufs=x_bufs) as xp,
            tc.tile_pool(name="op", bufs=o_bufs) as op,
        ):
            for t in range(nt * repeats):
                r0 = (t % nt) * tile_samples
                xt = xp.tile([P, spt, CIN, H, W], F32, name="xt")
                nc.sync.dma_start(out=xt, in_=x_d[r0 : r0 + tile_samples, :])

                ot = op.tile([P, spt, COUT, H, W], F32, name="ot")

                # identity channels on ScalarE (independent of everything else)
                nc.scalar.copy(out=ot[:, :, 0:CIN], in_=xt)

                # l=3 filters: DVE-only
                for i, (dh, dw) in enumerate(DIRS):
                    c0 = CIN + CIN * i
                    blk = ot[:, :, c0 : c0 + CIN]
                    hp, wp = _region(dh, dw)     # R+ (reads x[h+dh, w+dw])
                    hm, wm = _region(-dh, -dw)   # R-
                    # op1: fresh write covering R+
                    if dh == 0 or dw == 0:
                        o = blk[:, :, :, hp, wp]
                        nc.vector.tensor_add(
                            out=o,
                            in0=xt[:, :, :, hp, wp],
                            in1=xt[:, :, :, _shifted(hp, wp, dh, dw)[0],
                                  _shifted(hp, wp, dh, dw)[1]],
                        )
                    else:
                        for c in range(CIN):
                            shs, sws = _shifted(hp, wp, dh, dw)
                            nc.vector.tensor_add(
                                out=blk[:, :, c, hp, wp],
                                in0=xt[:, :, c, hp, wp],
                                in1=xt[:, :, c, shs, sws],
                            )
                    # op2 (+op2b): complement of R+ — fresh writes
                    if dh == 0:
                        # missing column(s): w = wp.stop..W-1 (single col for l3)
                        wc = W - 1 if dw > 0 else 0
                        nc.vector.tensor_add(
                            out=blk[:, :, :, :, wc],
                            in0=xt[:, :, :, :, wc],
                            in1=xt[:, :, :, :, wc - dw],
                        )
                    elif dw == 0:
                        hc = H - 1 if dh > 0 else 0
                        nc.vector.tensor_add(
                            out=blk[:, :, :, hc, :],
                            in0=xt[:, :, :, hc, :],
                            in1=xt[:, :, :, hc - dh, :],
                        )
                    else:
                        # L-shape: bottom row h=H-1 (since dh=1), plus column wc
                        hc = H - 1
                        wc = W - 1 if dw > 0 else 0
                        # bottom row cells with the -d tap: w such that w-dw in [0,W)
                        wr = slice(max(0, dw), W + min(0, dw))
                        nc.vector.tensor_add(
                            out=blk[:, :, :, hc, wr],
                            in0=xt[:, :, :, hc, wr],
                            in1=xt[:, :, :, hc - dh, slice(wr.start - dw, wr.stop - dw)],
                        )
                        # side column cells h in [1, H-1) with the -d tap
                        hr = slice(1, H - 1)
                        nc.vector.tensor_add(
                            out=blk[:, :, :, hr, wc],
                            in0=xt[:, :, :, hr, wc],
                            in1=xt[:, :, :, slice(hr.start - dh, hr.stop - dh), wc - dw],
                        )
                        # two corners: center tap only -> copies
                        nc.vector.tensor_copy(
                            out=blk[:, :, :, hc, W - 1 - wc],
                            in_=xt[:, :, :, hc, W - 1 - wc],
                        )
                        nc.vector.tensor_copy(
                            out=blk[:, :, :, 0, wc], in_=xt[:, :, :, 0, wc]
                        )
                    # op3: += x[-d] on R+ & R-
                    hi, wi = _isect(hp, hm), _isect(wp, wm)
                    shs, sws = _shifted(hi, wi, -dh, -dw)
                    if dh == 0 or dw == 0:
                        o = blk[:, :, :, hi, wi]
                        nc.vector.tensor_add(
                            out=o, in0=o, in1=xt[:, :, :, shs, sws]
                        )
                    else:
                        for c in range(CIN):
                            o = blk[:, :, c, hi, wi]
                            nc.vector.tensor_add(
                                out=o, in0=o, in1=xt[:, :, c, shs, sws]
                            )

                if split_out_dma:
                    lo = CIN + 12  # 15 channels: identity + l3
                    nc.sync.dma_start(
                        out=o_d[r0 : r0 + tile_samples, 0 : lo * H * W],
                        in_=ot[:, :, 0:lo],
                    )

                # l=5: ACT base copy of l3 result + two in-place DVE adds
                for i in range(4):
                    c0 = CIN + 12 + CIN * i
                    c3 = CIN + CIN * i
                    nc.scalar.copy(
                        out=ot[:, :, c0 : c0 + CIN], in_=ot[:, :, c3 : c3 + CIN]
                    )
                for i, (dh0, dw0) in enumerate(DIRS):
                    c0 = CIN + 12 + CIN * i
                    blk = ot[:, :, c0 : c0 + CIN]
                    for sgn in (2, -2):
                        dh, dw = sgn * dh0, sgn * dw0
                        hs, ws = _region(dh, dw)
                        shs, sws = _shifted(hs, ws, dh, dw)
                        if dh == 0 or dw == 0:
                            o = blk[:, :, :, hs, ws]
                            nc.vector.tensor_add(
                                out=o, in0=o, in1=xt[:, :, :, shs, sws]
                            )
                        else:
                            for c in range(CIN):
                                o = blk[:, :, c, hs, ws]
                                nc.vector.tensor_add(
                                    out=o, in0=o, in1=xt[:, :, c, shs, sws]
                                )

                if split_out_dma:
                    lo = CIN + 12
                    nc.sync.dma_start(
                        out=o_d[r0 : r0 + tile_samples, lo * H * W : FO],
                        in_=ot[:, :, lo:COUT],
                    )
                else:
                    nc.sync.dma_start(out=o_d[r0 : r0 + tile_samples, :], in_=ot)

    nc.compile()
    return nc


_NC_CACHE = {}


def _get_nc(n_samples):
    if n_samples not in _NC_CACHE:
        _NC_CACHE[n_samples] = build_nc(n_samples)
    return _NC_CACHE[n_samples]


def run(x, n_cores=N_CORES, trace=False, **spmd_kwargs):
    """Run the kernel on the first n_cores NeuronCores; x is the full batch."""
    x = np.ascontiguousarray(np.asarray(x, dtype=np.float32))
    n_total = x.shape[0]
    assert n_total % n_cores == 0
    per = n_total // n_cores
    flat = x.reshape(n_total, FX)
    shards = [np.ascontiguousarray(flat[i * per : (i + 1) * per]) for i in range(n_cores)]
    nc = _get_nc(per)
    in_maps = [{"x": s} for s in shards]
    res = run_bass_kernel_spmd(
        nc, in_maps, core_ids=list(range(n_cores)), trace=trace, **spmd_kwargs
    )
    out = np.concatenate([r["out"] for r in res.results], axis=0)
    return out.reshape(n_total, COUT, H, W), res


def kernel(x, **unused_filts):
    """Entry point: full inputs in, full output out. Filters are the fixed
    0/1 line patterns from the problem definition and are hardcoded."""
    out, _ = run(x)
    return out

